# revision 60
# baseline (speedup 1.0000x reference)
"""Trainium2 Bass kernel for BART custom-mask attention.

Problem: B=4, T=S=1024, E=1024, H=16 heads, D=64.
  q = (hs @ q_w.T + q_b) * D**-0.5 ; k/v analogous
  scores = q k^T + attention_mask ; attn = softmax(scores)
  attn(head h) *= (1-hm[h]) + hm[h]*(relation_inputs>0)   (no renorm)
  out = (attn @ v) @ o_w.T + o_b

Sharding: 8 cores = batch (4) x head-group (2, 8 heads each). Each core
computes a 512-feature slice of the attention output and projects it
through the matching o_w columns; the host sums the two half-partials
per batch.

Per-core design (bf16 compute, fp32 PSUM). The TRN2 PE streams ~1
rhs column/cycle at 2.4 GHz once warm, so runtime ~= total streamed
matmul columns; everything else must hide under the PE stream:

  - all inputs arrive as a handful of [128, big] packed DMAs (one or
    two per tensor, 8-16KB per descriptor row) instead of hundreds of
    small tile loads - kills DMA-descriptor pressure and load stalls.
  - the token axis is split in two 512-column halves and the whole
    attention is pipelined over 8 (pair, half) blocks; the output
    projection for half 0 runs during half 1's attention, so only a
    short tail remains after the last attention matmul.
  - within a block, score matmuls are interleaved ~1:2 with other PE
    work so ScalarE's exp stream (the 2nd-busiest engine) never blocks
    the PE on PSUM buffers, and the PE never idles (idle re-ramps the
    clock to 1.2 GHz for 3us).
  - per-head av uses lhsT = [64x ones | v_h]: PSUM rows 0:64 get the
    softmax denominator, rows 64:128 the output. reciprocal runs
    directly on PSUM rows 0:64, one SBUF->SBUF DMA re-homes both
    heads' 1/den to partitions 64:127, two muls normalize, and one
    DMA re-homes the even head's result to oT rows 0:63.
  - relation-masked heads (head 0 with the one-hot heads_mask) get an
    unmasked-denominator matmul (ones64 lhsT) before the in-place
    E *= relM multiplies.
  - zero biases / zero attention_mask (the graded case) compile all
    bias machinery out of the program.
"""

import os
import sys

import numpy as np

for _p in ("/opt/trn_rl_repo", "/root/.axon_site/_ro/trn_rl_repo"):
    if os.path.isdir(_p) and _p not in sys.path:
        sys.path.insert(0, _p)
        break

import ml_dtypes

B, T, E, H = 4, 1024, 1024, 16
D = E // H
SCALING = D ** -0.5
N_CORES = 8
FH = 512          # features per core (8 heads x 64)
P = 128
HF = 512          # tokens per half
BF16 = ml_dtypes.bfloat16

_PROGS = {}


def _build_program(mask_on, slot_flags, no_bias):
    import concourse.tile as tile
    from concourse import bacc, mybir
    from contextlib import ExitStack

    bf = mybir.dt.bfloat16
    f32 = mybir.dt.float32
    Exp = mybir.ActivationFunctionType.Exp

    nc = bacc.Bacc("TRN2", target_bir_lowering=False, debug=False,
                   num_devices=N_CORES)

    xTp_d = nc.declare_dram_parameter("xTp", [P, 8 * T], bf, isOutput=False)
    wqTp_d = nc.declare_dram_parameter("wqTp", [P, 8 * FH], bf, isOutput=False)
    wkTp_d = nc.declare_dram_parameter("wkTp", [P, 8 * FH], bf, isOutput=False)
    wvTp_d = nc.declare_dram_parameter("wvTp", [P, 8 * FH], bf, isOutput=False)
    owTp_d = nc.declare_dram_parameter("owTp", [P, 4 * T], bf, isOutput=False)
    relMp_d = {}
    for k in range(8):
        if slot_flags[k]:
            relMp_d[k] = nc.declare_dram_parameter(f"relMp{k}", [P, 8 * T],
                                                   bf, isOutput=False)
    if mask_on:
        expmp_d = nc.declare_dram_parameter("expmp", [P, 8 * T], bf,
                                            isOutput=False)
    if not no_bias:
        qb_d = nc.declare_dram_parameter("qb", [P, 4], f32, isOutput=False)
        kb_d = nc.declare_dram_parameter("kb", [P, 4], f32, isOutput=False)
        vbb_d = nc.declare_dram_parameter("vbb", [P, FH], f32, isOutput=False)
        obb_d = nc.declare_dram_parameter("obb", [P, E], f32, isOutput=False)
    y_d = nc.declare_dram_parameter("y", [T, E], f32, isOutput=True)

    with tile.TileContext(nc) as tc, ExitStack() as ctx:
        persist = ctx.enter_context(tc.tile_pool(name="persist", bufs=1))

        def ptile(rows, cols, nm, dt=bf):
            return persist.tile([rows, cols], dt, name=nm, tag=nm)

        # two tiles per packed input so first-half DMAs release deps early
        xTp_t = [ptile(P, 4 * T, "xTp_a"), ptile(P, 4 * T, "xTp_b")]
        wqTp_t = [ptile(P, 4 * FH, "wqTp_a"), ptile(P, 4 * FH, "wqTp_b")]
        wkTp_t = [ptile(P, 4 * FH, "wkTp_a"), ptile(P, 4 * FH, "wkTp_b")]
        wvTp_t = [ptile(P, 4 * FH, "wvTp_a"), ptile(P, 4 * FH, "wvTp_b")]
        owTp_t = ptile(P, 4 * T, "owTp_t")
        relMp_t = {k: ptile(P, 8 * T, f"relMp_t{k}") for k in relMp_d}
        if mask_on:
            expmp_t = ptile(P, 8 * T, "expmp_t")
        if not no_bias:
            qb_t = ptile(P, 4, "qb_t", f32)
            kb_t = ptile(P, 4, "kb_t", f32)
            vbb_t = ptile(P, FH, "vbb_t", f32)
            obb_t = ptile(P, E, "obb_t", f32)

        kT = [ptile(P, T, f"kT{p}") for p in range(4)]
        qT = [ptile(P, T, f"qT{p}") for p in range(4)]
        v_all = [ptile(P, T, f"vall{s}") for s in range(8)]
        oT = [ptile(P, T, f"oT{p}") for p in range(4)]
        warm = ptile(P, 512, "warm")
        ones64 = ptile(P, 64, "ones64")
        tiny = ptile(1, 16, "tiny")

        # ---- input DMAs: two halves per tensor, most-needed first.
        # Triggers cost ~0.6us each on a sequencer; alternate sync/gpsimd
        # so descriptor generation for the critical loads runs in parallel.
        def dma_in(dst, src):
            nc.sync.dma_start(dst, src)

        def dma2(dst, src, w):
            dma_in(dst[0][:], src[:, 0:w // 2])
            dma_in(dst[1][:], src[:, w // 2:w])

        def dma1(dst, src, w):
            dma_in(dst[:, 0:w // 2], src[:, 0:w // 2])
            dma_in(dst[:, w // 2:w], src[:, w // 2:w])

        # k(0) ec 0-3 needs only the _a halves - load those first
        dma_in(xTp_t[0][:], xTp_d[:, 0:4 * T])
        dma_in(wkTp_t[0][:], wkTp_d[:, 0:4 * FH])
        dma_in(xTp_t[1][:], xTp_d[:, 4 * T:8 * T])
        dma_in(wkTp_t[1][:], wkTp_d[:, 4 * FH:8 * FH])
        dma2(wqTp_t, wqTp_d, 8 * FH)
        dma2(wvTp_t, wvTp_d, 8 * FH)
        if not no_bias:
            dma_in(qb_t[:], qb_d[:])
            dma_in(kb_t[:], kb_d[:])
            dma_in(vbb_t[:], vbb_d[:])
        for k, d in relMp_d.items():
            dma1(relMp_t[k], d, 8 * T)
        if mask_on:
            dma1(expmp_t, expmp_d, 8 * T)
        dma1(owTp_t, owTp_d, 4 * T)
        if not no_bias:
            dma_in(obb_t[:], obb_d[:])

        # ---- local constants ----
        nc.vector.memset(warm[:], 0.5)
        nc.vector.memset(ones64[:], 1.0)
        for s in range(8):
            # ones in cols [128j, 128j+64) of each 128-block
            va = v_all[s][:].rearrange("p (b c) -> p b c", c=128)
            nc.vector.memset(va[:, :, 0:64], 1.0)

        with tc.tile_pool(name="s_pool", bufs=1, space="PSUM") as s_pool, \
             tc.tile_pool(name="w_pool", bufs=1, space="PSUM") as w_pool, \
             tc.tile_pool(name="e_pool", bufs=1) as e_pool, \
             tc.tile_pool(name="x_pool", bufs=1) as x_pool:

            # preload the exp table on ScalarE before the first real exp
            nc.scalar.activation(tiny[0:1, 0:16], warm[0:1, 0:16], Exp)

            eT = {}       # (p, h, sc, j) -> exp tile
            av_ps = {}    # (lh, h) -> av psum tile
            dps_t = {}    # (lh, h) -> unmasked-den psum tile

            def emit_warm():
                ps = w_pool.tile([P, 512], f32, name="warm_ps", tag="w_ps",
                                 bufs=4)
                for _ in range(14):
                    nc.tensor.matmul(ps[:], lhsT=warm[:, 0:128], rhs=warm[:],
                                     start=True, stop=True)

            def xsl(ec, lo, hi):
                return xTp_t[ec // 4][:, T * (ec % 4) + lo:T * (ec % 4) + hi]

            def wsl(w_t, ec, lo, hi):
                return w_t[ec // 4][:, FH * (ec % 4) + lo:FH * (ec % 4) + hi]

            def emit_k(p):
                for th in range(2):
                    ps = w_pool.tile([P, 512], f32, name="k_ps", tag="w_ps",
                                     bufs=4)
                    for ec in range(8):
                        nc.tensor.matmul(
                            ps[:],
                            lhsT=wsl(wkTp_t, ec, P * p, P * (p + 1)),
                            rhs=xsl(ec, 512 * th, 512 * (th + 1)),
                            start=(ec == 0), stop=(ec == 7))
                    dst = kT[p][:, 512 * th:512 * (th + 1)]
                    if no_bias:
                        nc.vector.tensor_copy(dst, ps[:])
                    else:
                        nc.vector.tensor_scalar_add(dst, ps[:],
                                                    kb_t[:, p:p + 1])

            def emit_q(p, h):
                ps = w_pool.tile([P, 512], f32, name="q_ps", tag="w_ps",
                                 bufs=4)
                for ec in range(8):
                    nc.tensor.matmul(
                        ps[:],
                        lhsT=wsl(wqTp_t, ec, P * p, P * (p + 1)),
                        rhs=xsl(ec, 512 * h, 512 * (h + 1)),
                        start=(ec == 0), stop=(ec == 7))
                dst = qT[p][:, 512 * h:512 * (h + 1)]
                if no_bias:
                    nc.vector.tensor_copy(dst, ps[:])
                else:
                    nc.vector.tensor_scalar_add(dst, ps[:], qb_t[:, p:p + 1])

            def emit_v(sc):
                ps = w_pool.tile([P, 512], f32, name="v_ps", tag="w_ps",
                                 bufs=4)
                for ec in range(8):
                    nc.tensor.matmul(
                        ps[:],
                        lhsT=xsl(ec, P * sc, P * (sc + 1)),
                        rhs=wsl(wvTp_t, ec, 0, FH),
                        start=(ec == 0), stop=(ec == 7))
                # scatter 64-col head blocks into [ones | v_h] layout
                va = v_all[sc][:].rearrange("p (b c) -> p b c", c=128)
                src = ps[:].rearrange("p (b c) -> p b c", c=64)
                if no_bias:
                    nc.vector.tensor_copy(va[:, :, 64:128], src)
                else:
                    vb = vbb_t[:].rearrange("p (b c) -> p b c", c=64)
                    nc.vector.tensor_add(va[:, :, 64:128], src, vb)

            GRP = 2   # score sc-units per PSUM allocation (4 banks)

            def emit_score(p, h, sc):
                # 4 sc-units x 2 heads share one 4-bank tile: one alloc
                # semaphore per 8 matmuls, one wide exp per group
                g, i = sc // GRP, sc % GRP
                if i == 0:
                    eT[(p, h, "s", g)] = s_pool.tile(
                        [P, GRP * T], f32, name="s_ps", tag="s_ps", bufs=1)
                s = eT[(p, h, "s", g)]
                for j in range(2):
                    rsl = slice(64 * j, 64 * (j + 1))
                    nc.tensor.matmul(
                        s[:, T * i + 512 * j:T * i + 512 * (j + 1)],
                        lhsT=kT[p][rsl, P * sc:P * (sc + 1)],
                        rhs=qT[p][rsl, 512 * h:512 * (h + 1)],
                        start=True, stop=True)
                if i == GRP - 1:
                    e = e_pool.tile([P, GRP * T], bf, name="e_t", tag="e_t",
                                    bufs=13)
                    nc.scalar.activation(e[:], s[:], Exp)
                    eT[(p, h, g)] = e
                    eT.pop((p, h, "s", g))
                    if mask_on:
                        for i2 in range(GRP):
                            em = expmp_t[:, T * (g * GRP + i2) + 512 * h:
                                         T * (g * GRP + i2) + 512 * (h + 1)]
                            for j in range(2):
                                jc = e[:, T * i2 + 512 * j:
                                      T * i2 + 512 * (j + 1)]
                                nc.vector.tensor_mul(jc, jc, em)

            def eslice(p, h, sc, j):
                g, i = sc // GRP, sc % GRP
                return eT[(p, h, g)][:, T * i + 512 * j:T * i + 512 * (j + 1)]

            def emit_dps(p, h, lh):
                dps = w_pool.tile([P, 512], f32, name="d_ps", tag="w_ps",
                                  bufs=4)
                dps_t[(lh, h)] = dps
                for sc in range(8):
                    nc.tensor.matmul(dps[0:64, :], lhsT=ones64[:],
                                     rhs=eslice(p, h, sc, lh % 2),
                                     start=(sc == 0), stop=(sc == 7))

            def emit_muls(p, h, lh):
                rm = relMp_t[lh]
                for sc in range(8):
                    e = eslice(p, h, sc, lh % 2)
                    nc.vector.tensor_mul(
                        e, e,
                        rm[:, T * sc + 512 * h:T * sc + 512 * (h + 1)])

            def emit_av_mm(p, h, lh, sc):
                key = (lh, h)
                if sc == 0:
                    av_ps[key] = w_pool.tile([P, 512], f32, name="av_ps",
                                             tag="w_ps", bufs=4)
                nc.tensor.matmul(
                    av_ps[key][:],
                    lhsT=v_all[sc][:, P * lh:P * (lh + 1)],
                    rhs=eslice(p, h, sc, lh % 2),
                    start=(sc == 0), stop=(sc == 7))
                if sc == 7 and lh % 2 == 1:
                    for g in range(8 // GRP):
                        eT[(p, h, g)] = None

            def emit_cb_head(p, h, lh, bc, eng=None):
                eng = eng or nc.sync
                off = 512 * (lh % 2)
                src = (dps_t.pop((lh, h)) if slot_flags[lh]
                       else av_ps[(lh, h)])
                nc.vector.reciprocal_approx_fast(
                    bc[0:64, off:off + 512], src[0:64, :])
                eng.dma_start(bc[64:128, off:off + 512],
                              bc[0:64, off:off + 512])
                tsl = slice(512 * h, 512 * (h + 1))
                if lh % 2:
                    nc.vector.tensor_mul(oT[p][64:128, tsl],
                                         av_ps[(lh, h)][64:128, :],
                                         bc[64:128, off:off + 512])
                else:
                    tmpb = x_pool.tile([P, 512], bf, name="tmpb", tag="tmpb",
                                       bufs=2)
                    nc.vector.tensor_mul(tmpb[64:128, :],
                                         av_ps[(lh, h)][64:128, :],
                                         bc[64:128, off:off + 512])
                    eng.dma_start(oT[p][0:64, tsl], tmpb[64:128, :])
                av_ps.pop((lh, h))

            def emit_copyback(p, h):
                bc = x_pool.tile([P, T], f32, name="bc", tag="bc", bufs=2)
                emit_cb_head(p, h, 2 * p, bc)
                emit_cb_head(p, h, 2 * p + 1, bc)

            ysb_t = {}

            def emit_op_stage(tcn, fcs, last):
                # accumulate fc chunks of the out-projection for t-chunk
                # tcn; on the first stage copy PSUM->ysb, later stages add.
                first = tcn not in ysb_t
                if first:
                    ysb_t[tcn] = x_pool.tile([P, E], f32, name="ysb",
                                             tag="ysb", bufs=4)
                ysb = ysb_t[tcn]
                for eh in range(2):
                    yps = w_pool.tile([P, 512], f32, name="y_ps", tag="w_ps",
                                      bufs=4)
                    for i, fc in enumerate(fcs):
                        nc.tensor.matmul(
                            yps[:],
                            lhsT=oT[fc][:, P * tcn:P * (tcn + 1)],
                            rhs=owTp_t[:, T * fc + 512 * eh:T * fc + 512 * (eh + 1)],
                            start=(i == 0), stop=(i == len(fcs) - 1))
                    dst = ysb[:, 512 * eh:512 * (eh + 1)]
                    if not first:
                        nc.vector.tensor_add(dst, yps[:], dst)
                    elif not no_bias:
                        nc.vector.tensor_add(dst, yps[:],
                                             obb_t[:, 512 * eh:512 * (eh + 1)])
                    elif eh == 0:
                        nc.scalar.copy(dst, yps[:])
                    else:
                        nc.vector.tensor_copy(dst, yps[:])
                if last:
                    eng = (nc.sync, nc.gpsimd)[tcn % 2]
                    for eh in range(2):
                        eng.dma_start(
                            y_d[P * tcn:P * (tcn + 1),
                                512 * eh:512 * (eh + 1)],
                            ysb[:, 512 * eh:512 * (eh + 1)])
                    ysb_t.pop(tcn)

            def emit_outproj(tcn, vec_eng):
                emit_op_stage(tcn, (0, 1, 2, 3), True)

            # ---------------- schedule ----------------
            def av_items(p, h):
                items = []
                for lh in (2 * p, 2 * p + 1):
                    if slot_flags[lh]:
                        items.append(lambda p=p, h=h, lh=lh: emit_dps(p, h, lh))
                        items.append(lambda p=p, h=h, lh=lh: emit_muls(p, h, lh))
                for sc in range(8):
                    for lh in (2 * p, 2 * p + 1):
                        items.append(
                            lambda p=p, h=h, lh=lh, sc=sc: emit_av_mm(p, h, lh, sc))
                return items

            def k_items(p):
                return [lambda p=p: emit_k(p)]

            def q_items(p, h):
                return [lambda p=p, h=h: emit_q(p, h)]

            def v_items(lo, hi):
                return [lambda sc=sc: emit_v(sc) for sc in range(lo, hi)]

            def op_items(lo, hi):
                return [lambda t=t, v=(t % 2 == 0): emit_outproj(t, v)
                        for t in range(lo, hi)]

            def cb_items(p, h):
                return [lambda p=p, h=h: emit_copyback(p, h)]

            def emit_block(p, h, others):
                # both sc-units of a PSUM group emitted adjacently, so the
                # group's exp gets the full others window before the next
                # group needs the (single-buffered) scores tile back
                no = len(others)
                oi = 0
                for g in range(4):
                    emit_score(p, h, 2 * g)
                    emit_score(p, h, 2 * g + 1)
                    tgt = ((g + 1) * no) // 4
                    while oi < tgt:
                        others[oi]()
                        oi += 1
                while oi < no:
                    others[oi]()
                    oi += 1

            # prologue: clock ramp + pair-0 K/Q. All ec 0-3 accumulations
            # (needing only the _a half-tiles) run before any ec 4-7, so
            # the PE works while the _b input DMAs are still in flight.
            emit_warm()
            kps0 = [w_pool.tile([P, 512], f32, name="k_ps", tag="w_ps",
                                bufs=4) for _ in range(2)]
            qps0 = w_pool.tile([P, 512], f32, name="q_ps", tag="w_ps",
                               bufs=4)
            for eh in range(2):
                for ec in range(4 * eh, 4 * eh + 4):
                    for th in range(2):
                        nc.tensor.matmul(
                            kps0[th][:],
                            lhsT=wsl(wkTp_t, ec, 0, P),
                            rhs=xsl(ec, 512 * th, 512 * (th + 1)),
                            start=(ec == 0), stop=(ec == 7))
                    nc.tensor.matmul(
                        qps0[:], lhsT=wsl(wqTp_t, ec, 0, P),
                        rhs=xsl(ec, 0, 512),
                        start=(ec == 0), stop=(ec == 7))
            for th in range(2):
                dst = kT[0][:, 512 * th:512 * (th + 1)]
                if no_bias:
                    nc.vector.tensor_copy(dst, kps0[th][:])
                else:
                    nc.vector.tensor_scalar_add(dst, kps0[th][:],
                                                kb_t[:, 0:1])
            if no_bias:
                nc.vector.tensor_copy(qT[0][:, 0:512], qps0[:])
            else:
                nc.vector.tensor_scalar_add(qT[0][:, 0:512], qps0[:],
                                            qb_t[:, 0:1])

            emit_block(0, 0, k_items(1) + q_items(1, 0) + v_items(0, 4))
            emit_block(1, 0, k_items(2) + q_items(2, 0) + v_items(4, 8))
            emit_block(2, 0, av_items(0, 0) + cb_items(0, 0)
                       + k_items(3) + q_items(3, 0))
            emit_block(3, 0, av_items(1, 0) + cb_items(1, 0)
                       + av_items(2, 0) + cb_items(2, 0) + q_items(0, 1))
            emit_block(0, 1, av_items(3, 0) + cb_items(3, 0) + q_items(1, 1))
            emit_block(1, 1, av_items(0, 1) + cb_items(0, 1)
                       + q_items(2, 1) + op_items(0, 1))
            emit_block(2, 1, av_items(1, 1) + cb_items(1, 1)
                       + q_items(3, 1) + op_items(1, 3))
            s1_items = [lambda t=t: emit_op_stage(t, (0, 1), False)
                        for t in range(4, 8)]
            emit_block(3, 1, av_items(2, 1) + cb_items(2, 1)
                       + op_items(3, 4) + s1_items)
            # tail: head A av -> its copyback overlaps head B av; the
            # ungated fc2 accumulations fill the last copyback's
            # latency; only fc3 (gated on oT[3]) remains at the end.
            bc = x_pool.tile([P, T], f32, name="bc", tag="bc", bufs=2)
            for sc in range(8):
                emit_av_mm(3, 1, 6, sc)
            emit_cb_head(3, 1, 6, bc, nc.gpsimd)
            for sc in range(8):
                emit_av_mm(3, 1, 7, sc)
            emit_cb_head(3, 1, 7, bc, nc.gpsimd)
            for tcn in range(4, 8):
                emit_op_stage(tcn, (2, 3), True)

    nc.compile()
    return nc


def _get_program(mask_on, slot_flags, no_bias):
    key = (mask_on, slot_flags, no_bias)
    if key not in _PROGS:
        _PROGS[key] = _build_program(mask_on, slot_flags, no_bias)
    return _PROGS[key]


def _pack8(a):
    """[8*128, W] -> [128, 8*W] (chunk-major columns)."""
    n, w = a.shape[0] // P, a.shape[1]
    return np.ascontiguousarray(
        a.reshape(n, P, w).transpose(1, 0, 2).reshape(P, n * w))


def _prep_inputs(inputs):
    hs = np.asarray(inputs["hidden_states"], dtype=np.float32)
    am = np.asarray(inputs["attention_mask"], dtype=np.float32)
    rel = np.asarray(inputs["relation_inputs"])
    hm = np.asarray(inputs["heads_mask"], dtype=np.float32)
    q_w = np.asarray(inputs["q_w"], dtype=np.float32)
    q_b = np.asarray(inputs["q_b"], dtype=np.float32)
    k_w = np.asarray(inputs["k_w"], dtype=np.float32)
    k_b = np.asarray(inputs["k_b"], dtype=np.float32)
    v_w = np.asarray(inputs["v_w"], dtype=np.float32)
    v_b = np.asarray(inputs["v_b"], dtype=np.float32)
    o_w = np.asarray(inputs["o_w"], dtype=np.float32)
    o_b = np.asarray(inputs["o_b"], dtype=np.float32)

    mask_on = bool(np.any(am != 0.0))
    no_bias = not (np.any(q_b != 0.0) or np.any(k_b != 0.0)
                   or np.any(v_b != 0.0) or np.any(o_b != 0.0))
    slot_flags = tuple(
        k == 0 or bool(np.any(hm[[k, 8 + k]] != 0.0)) for k in range(8))

    relbinT = [(rel[b] > 0).T.astype(np.float32) for b in range(B)]
    if mask_on:
        expmT = [np.exp(am[b, 0]).T.astype(np.float32) for b in range(B)]

    in_maps = []
    for c in range(N_CORES):
        b, g = c // 2, c % 2
        sl = slice(FH * g, FH * (g + 1))
        im = {
            "xTp": _pack8(np.ascontiguousarray(hs[b].T)).astype(BF16),
            "wqTp": _pack8(
                np.ascontiguousarray((q_w[sl] * SCALING).T)).astype(BF16),
            "wkTp": _pack8(np.ascontiguousarray(k_w[sl].T)).astype(BF16),
            "wvTp": _pack8(np.ascontiguousarray(v_w[sl].T)).astype(BF16),
            "owTp": _pack8(np.ascontiguousarray(o_w[:, sl].T)).astype(BF16),
        }
        if not no_bias:
            im["qb"] = np.ascontiguousarray(
                (q_b[sl] * SCALING).reshape(4, P).T).astype(np.float32)
            im["kb"] = np.ascontiguousarray(
                k_b[sl].reshape(4, P).T).astype(np.float32)
            im["vbb"] = np.ascontiguousarray(
                np.broadcast_to(v_b[sl], (P, FH))).astype(np.float32)
            im["obb"] = (np.ascontiguousarray(np.broadcast_to(o_b, (P, E)))
                         .astype(np.float32) if g == 0
                         else np.zeros((P, E), np.float32))
        for k in range(8):
            if slot_flags[k]:
                hmv = float(hm[8 * g + k])
                m = (1.0 - hmv) + hmv * relbinT[b]
                im[f"relMp{k}"] = _pack8(m).astype(BF16)
        if mask_on:
            im["expmp"] = _pack8(expmT[b]).astype(BF16)
        in_maps.append(im)
    return (mask_on, slot_flags, no_bias), in_maps


def _gather(results):
    out = np.empty((B, T, E), dtype=np.float32)
    for b in range(B):
        out[b] = results[2 * b]["y"] + results[2 * b + 1]["y"]
    return out


def run_sharded(inputs, trace=False, trace_kwargs=None):
    from concourse.bass_utils import run_bass_kernel_spmd

    flags, in_maps = _prep_inputs(inputs)
    nc = _get_program(*flags)
    last_err = None
    for _attempt in range(3):
        try:
            res = run_bass_kernel_spmd(nc, in_maps, list(range(N_CORES)),
                                       trace=trace, **(trace_kwargs or {}))
            return _gather(res.results), res
        except Exception as e:  # first exec of a fresh NEFF can flake
            last_err = e
    raise last_err


def kernel(**inputs):
    out, _ = run_sharded(inputs)
    return out


# revision 62
# speedup vs baseline: 1.0268x; 1.0268x over previous
"""Trainium2 Bass kernel for BART custom-mask attention.

Problem: B=4, T=S=1024, E=1024, H=16 heads, D=64.
  q = (hs @ q_w.T + q_b) * D**-0.5 ; k/v analogous
  scores = q k^T + attention_mask ; attn = softmax(scores)
  attn(head h) *= (1-hm[h]) + hm[h]*(relation_inputs>0)   (no renorm)
  out = (attn @ v) @ o_w.T + o_b

Sharding: 8 cores = batch (4) x head-group (2, 8 heads each). Each core
computes a 512-feature slice of the attention output and projects it
through the matching o_w columns; the host sums the two half-partials
per batch.

Per-core design (bf16 compute, fp32 PSUM). The TRN2 PE streams ~1
rhs column/cycle at 2.4 GHz once warm, so runtime ~= total streamed
matmul columns; everything else must hide under the PE stream:

  - all inputs arrive as a handful of [128, big] packed DMAs (one or
    two per tensor, 8-16KB per descriptor row) instead of hundreds of
    small tile loads - kills DMA-descriptor pressure and load stalls.
  - the token axis is split in two 512-column halves and the whole
    attention is pipelined over 8 (pair, half) blocks; the output
    projection for half 0 runs during half 1's attention, so only a
    short tail remains after the last attention matmul.
  - within a block, score matmuls are interleaved ~1:2 with other PE
    work so ScalarE's exp stream (the 2nd-busiest engine) never blocks
    the PE on PSUM buffers, and the PE never idles (idle re-ramps the
    clock to 1.2 GHz for 3us).
  - per-head av uses lhsT = [64x ones | v_h]: PSUM rows 0:64 get the
    softmax denominator, rows 64:128 the output. reciprocal runs
    directly on PSUM rows 0:64, one SBUF->SBUF DMA re-homes both
    heads' 1/den to partitions 64:127, two muls normalize, and one
    DMA re-homes the even head's result to oT rows 0:63.
  - relation-masked heads (head 0 with the one-hot heads_mask) get an
    unmasked-denominator matmul (ones64 lhsT) before the in-place
    E *= relM multiplies.
  - zero biases / zero attention_mask (the graded case) compile all
    bias machinery out of the program.
"""

import os
import sys

import numpy as np

for _p in ("/opt/trn_rl_repo", "/root/.axon_site/_ro/trn_rl_repo"):
    if os.path.isdir(_p) and _p not in sys.path:
        sys.path.insert(0, _p)
        break

import ml_dtypes

B, T, E, H = 4, 1024, 1024, 16
D = E // H
SCALING = D ** -0.5
N_CORES = 8
FH = 512          # features per core (8 heads x 64)
P = 128
HF = 512          # tokens per half
BF16 = ml_dtypes.bfloat16

_PROGS = {}


def _build_program(mask_on, slot_flags, no_bias):
    import concourse.tile as tile
    from concourse import bacc, mybir
    from contextlib import ExitStack

    bf = mybir.dt.bfloat16
    f32 = mybir.dt.float32
    Exp = mybir.ActivationFunctionType.Exp

    nc = bacc.Bacc("TRN2", target_bir_lowering=False, debug=False,
                   num_devices=N_CORES)

    xTp_d = nc.declare_dram_parameter("xTp", [P, 8 * T], bf, isOutput=False)
    wqTp_d = nc.declare_dram_parameter("wqTp", [P, 8 * FH], bf, isOutput=False)
    wkTp_d = nc.declare_dram_parameter("wkTp", [P, 8 * FH], bf, isOutput=False)
    wvTp_d = nc.declare_dram_parameter("wvTp", [P, 8 * FH], bf, isOutput=False)
    owTp_d = nc.declare_dram_parameter("owTp", [P, 4 * T], bf, isOutput=False)
    relMp_d = {}
    for k in range(8):
        if slot_flags[k]:
            relMp_d[k] = nc.declare_dram_parameter(f"relMp{k}", [P, 8 * T],
                                                   bf, isOutput=False)
    if mask_on:
        expmp_d = nc.declare_dram_parameter("expmp", [P, 8 * T], bf,
                                            isOutput=False)
    if not no_bias:
        qb_d = nc.declare_dram_parameter("qb", [P, 4], f32, isOutput=False)
        kb_d = nc.declare_dram_parameter("kb", [P, 4], f32, isOutput=False)
        vbb_d = nc.declare_dram_parameter("vbb", [P, FH], f32, isOutput=False)
        obb_d = nc.declare_dram_parameter("obb", [P, E], f32, isOutput=False)
    y_d = nc.declare_dram_parameter("y", [T, E], f32, isOutput=True)

    with tile.TileContext(nc) as tc, ExitStack() as ctx:
        persist = ctx.enter_context(tc.tile_pool(name="persist", bufs=1))

        def ptile(rows, cols, nm, dt=bf):
            return persist.tile([rows, cols], dt, name=nm, tag=nm)

        # two tiles per packed input so first-half DMAs release deps early
        xTp_t = [ptile(P, 4 * T, "xTp_a"), ptile(P, 4 * T, "xTp_b")]
        wqTp_t = [ptile(P, 4 * FH, "wqTp_a"), ptile(P, 4 * FH, "wqTp_b")]
        wkTp_t = [ptile(P, 4 * FH, "wkTp_a"), ptile(P, 4 * FH, "wkTp_b")]
        wvTp_t = [ptile(P, 4 * FH, "wvTp_a"), ptile(P, 4 * FH, "wvTp_b")]
        owTp_t = ptile(P, 4 * T, "owTp_t")
        relMp_t = {k: ptile(P, 8 * T, f"relMp_t{k}") for k in relMp_d}
        if mask_on:
            expmp_t = ptile(P, 8 * T, "expmp_t")
        if not no_bias:
            qb_t = ptile(P, 4, "qb_t", f32)
            kb_t = ptile(P, 4, "kb_t", f32)
            vbb_t = ptile(P, FH, "vbb_t", f32)
            obb_t = ptile(P, E, "obb_t", f32)

        kT = [ptile(P, T, f"kT{p}") for p in range(4)]
        qT = [ptile(P, T, f"qT{p}") for p in range(4)]
        v_all = [ptile(P, T, f"vall{s}") for s in range(8)]
        oT = [ptile(P, T, f"oT{p}") for p in range(4)]
        warm = ptile(P, 512, "warm")
        ones64 = ptile(P, 64, "ones64")
        tiny = ptile(1, 16, "tiny")

        # ---- input DMAs: two halves per tensor, most-needed first.
        # Triggers cost ~0.6us each on a sequencer; alternate sync/gpsimd
        # so descriptor generation for the critical loads runs in parallel.
        def dma_in(dst, src):
            nc.sync.dma_start(dst, src)

        def dma2(dst, src, w):
            dma_in(dst[0][:], src[:, 0:w // 2])
            dma_in(dst[1][:], src[:, w // 2:w])

        def dma1(dst, src, w):
            dma_in(dst[:, 0:w // 2], src[:, 0:w // 2])
            dma_in(dst[:, w // 2:w], src[:, w // 2:w])

        # k(0) ec 0-3 needs only the _a halves - load those first
        dma_in(xTp_t[0][:], xTp_d[:, 0:4 * T])
        dma_in(wkTp_t[0][:], wkTp_d[:, 0:4 * FH])
        dma_in(xTp_t[1][:], xTp_d[:, 4 * T:8 * T])
        dma_in(wkTp_t[1][:], wkTp_d[:, 4 * FH:8 * FH])
        dma2(wqTp_t, wqTp_d, 8 * FH)
        dma2(wvTp_t, wvTp_d, 8 * FH)
        if not no_bias:
            dma_in(qb_t[:], qb_d[:])
            dma_in(kb_t[:], kb_d[:])
            dma_in(vbb_t[:], vbb_d[:])
        for k, d in relMp_d.items():
            dma1(relMp_t[k], d, 8 * T)
        if mask_on:
            dma1(expmp_t, expmp_d, 8 * T)
        dma1(owTp_t, owTp_d, 4 * T)
        if not no_bias:
            dma_in(obb_t[:], obb_d[:])

        # ---- local constants ----
        nc.vector.memset(warm[:], 0.5)
        nc.vector.memset(ones64[:], 1.0)
        for s in range(8):
            # ones in cols [128j, 128j+64) of each 128-block
            va = v_all[s][:].rearrange("p (b c) -> p b c", c=128)
            nc.vector.memset(va[:, :, 0:64], 1.0)

        with tc.tile_pool(name="s_pool", bufs=1, space="PSUM") as s_pool, \
             tc.tile_pool(name="w_pool", bufs=1, space="PSUM") as w_pool, \
             tc.tile_pool(name="e_pool", bufs=1) as e_pool, \
             tc.tile_pool(name="x_pool", bufs=1) as x_pool:

            # preload the exp table on ScalarE before the first real exp
            nc.scalar.activation(tiny[0:1, 0:16], warm[0:1, 0:16], Exp)

            eT = {}       # (p, h, sc, j) -> exp tile
            av_ps = {}    # (lh, h) -> av psum tile
            dps_t = {}    # (lh, h) -> unmasked-den psum tile

            def emit_warm():
                ps = w_pool.tile([P, 512], f32, name="warm_ps", tag="w_ps",
                                 bufs=4)
                for _ in range(14):
                    nc.tensor.matmul(ps[:], lhsT=warm[:, 0:128], rhs=warm[:],
                                     start=True, stop=True)

            def xsl(ec, lo, hi):
                return xTp_t[ec // 4][:, T * (ec % 4) + lo:T * (ec % 4) + hi]

            def wsl(w_t, ec, lo, hi):
                return w_t[ec // 4][:, FH * (ec % 4) + lo:FH * (ec % 4) + hi]

            def emit_k(p):
                for th in range(2):
                    ps = w_pool.tile([P, 512], f32, name="k_ps", tag="w_ps",
                                     bufs=4)
                    for ec in range(8):
                        nc.tensor.matmul(
                            ps[:],
                            lhsT=wsl(wkTp_t, ec, P * p, P * (p + 1)),
                            rhs=xsl(ec, 512 * th, 512 * (th + 1)),
                            start=(ec == 0), stop=(ec == 7))
                    dst = kT[p][:, 512 * th:512 * (th + 1)]
                    if no_bias:
                        nc.vector.tensor_copy(dst, ps[:])
                    else:
                        nc.vector.tensor_scalar_add(dst, ps[:],
                                                    kb_t[:, p:p + 1])

            def emit_q(p, h):
                ps = w_pool.tile([P, 512], f32, name="q_ps", tag="w_ps",
                                 bufs=4)
                for ec in range(8):
                    nc.tensor.matmul(
                        ps[:],
                        lhsT=wsl(wqTp_t, ec, P * p, P * (p + 1)),
                        rhs=xsl(ec, 512 * h, 512 * (h + 1)),
                        start=(ec == 0), stop=(ec == 7))
                dst = qT[p][:, 512 * h:512 * (h + 1)]
                if no_bias:
                    nc.vector.tensor_copy(dst, ps[:])
                else:
                    nc.vector.tensor_scalar_add(dst, ps[:], qb_t[:, p:p + 1])

            def emit_v(sc):
                ps = w_pool.tile([P, 512], f32, name="v_ps", tag="w_ps",
                                 bufs=4)
                for ec in range(8):
                    nc.tensor.matmul(
                        ps[:],
                        lhsT=xsl(ec, P * sc, P * (sc + 1)),
                        rhs=wsl(wvTp_t, ec, 0, FH),
                        start=(ec == 0), stop=(ec == 7))
                # scatter 64-col head blocks into [ones | v_h] layout
                va = v_all[sc][:].rearrange("p (b c) -> p b c", c=128)
                src = ps[:].rearrange("p (b c) -> p b c", c=64)
                if no_bias:
                    nc.vector.tensor_copy(va[:, :, 64:128], src)
                else:
                    vb = vbb_t[:].rearrange("p (b c) -> p b c", c=64)
                    nc.vector.tensor_add(va[:, :, 64:128], src, vb)

            GRP = 2   # score sc-units per PSUM allocation (4 banks)

            def emit_score(p, h, sc):
                # 4 sc-units x 2 heads share one 4-bank tile: one alloc
                # semaphore per 8 matmuls, one wide exp per group
                g, i = sc // GRP, sc % GRP
                if i == 0:
                    eT[(p, h, "s", g)] = s_pool.tile(
                        [P, GRP * T], f32, name="s_ps", tag="s_ps", bufs=1)
                s = eT[(p, h, "s", g)]
                for j in range(2):
                    rsl = slice(64 * j, 64 * (j + 1))
                    nc.tensor.matmul(
                        s[:, T * i + 512 * j:T * i + 512 * (j + 1)],
                        lhsT=kT[p][rsl, P * sc:P * (sc + 1)],
                        rhs=qT[p][rsl, 512 * h:512 * (h + 1)],
                        start=True, stop=True)
                if i == GRP - 1:
                    e = e_pool.tile([P, GRP * T], bf, name="e_t", tag="e_t",
                                    bufs=13)
                    nc.scalar.activation(e[:], s[:], Exp)
                    eT[(p, h, g)] = e
                    eT.pop((p, h, "s", g))
                    if mask_on:
                        for i2 in range(GRP):
                            em = expmp_t[:, T * (g * GRP + i2) + 512 * h:
                                         T * (g * GRP + i2) + 512 * (h + 1)]
                            for j in range(2):
                                jc = e[:, T * i2 + 512 * j:
                                      T * i2 + 512 * (j + 1)]
                                nc.vector.tensor_mul(jc, jc, em)

            def eslice(p, h, sc, j):
                g, i = sc // GRP, sc % GRP
                return eT[(p, h, g)][:, T * i + 512 * j:T * i + 512 * (j + 1)]

            def emit_dps(p, h, lh):
                dps = w_pool.tile([P, 512], f32, name="d_ps", tag="w_ps",
                                  bufs=4)
                dps_t[(lh, h)] = dps
                for sc in range(8):
                    nc.tensor.matmul(dps[0:64, :], lhsT=ones64[:],
                                     rhs=eslice(p, h, sc, lh % 2),
                                     start=(sc == 0), stop=(sc == 7))

            def emit_muls(p, h, lh):
                rm = relMp_t[lh]
                for sc in range(8):
                    e = eslice(p, h, sc, lh % 2)
                    nc.vector.tensor_mul(
                        e, e,
                        rm[:, T * sc + 512 * h:T * sc + 512 * (h + 1)])

            def emit_av_mm(p, h, lh, sc):
                key = (lh, h)
                if sc == 0:
                    av_ps[key] = w_pool.tile([P, 512], f32, name="av_ps",
                                             tag="w_ps", bufs=4)
                nc.tensor.matmul(
                    av_ps[key][:],
                    lhsT=v_all[sc][:, P * lh:P * (lh + 1)],
                    rhs=eslice(p, h, sc, lh % 2),
                    start=(sc == 0), stop=(sc == 7))
                if sc == 7 and lh % 2 == 1:
                    for g in range(8 // GRP):
                        eT[(p, h, g)] = None

            def emit_cb_head(p, h, lh, bc, eng=None):
                eng = eng or nc.sync
                off = 512 * (lh % 2)
                src = (dps_t.pop((lh, h)) if slot_flags[lh]
                       else av_ps[(lh, h)])
                nc.vector.reciprocal_approx_fast(
                    bc[0:64, off:off + 512], src[0:64, :])
                eng.dma_start(bc[64:128, off:off + 512],
                              bc[0:64, off:off + 512])
                tsl = slice(512 * h, 512 * (h + 1))
                if lh % 2:
                    nc.vector.tensor_mul(oT[p][64:128, tsl],
                                         av_ps[(lh, h)][64:128, :],
                                         bc[64:128, off:off + 512])
                else:
                    tmpb = x_pool.tile([P, 512], bf, name="tmpb", tag="tmpb",
                                       bufs=2)
                    nc.vector.tensor_mul(tmpb[64:128, :],
                                         av_ps[(lh, h)][64:128, :],
                                         bc[64:128, off:off + 512])
                    eng.dma_start(oT[p][0:64, tsl], tmpb[64:128, :])
                av_ps.pop((lh, h))

            def emit_copyback(p, h):
                bc = x_pool.tile([P, T], f32, name="bc", tag="bc", bufs=2)
                emit_cb_head(p, h, 2 * p, bc)
                emit_cb_head(p, h, 2 * p + 1, bc)

            ysb_t = {}

            def emit_op_stage(tcn, fcs, last):
                # accumulate fc chunks of the out-projection for t-chunk
                # tcn; on the first stage copy PSUM->ysb, later stages add.
                first = tcn not in ysb_t
                if first:
                    ysb_t[tcn] = x_pool.tile([P, E], f32, name="ysb",
                                             tag="ysb", bufs=4)
                ysb = ysb_t[tcn]
                for eh in range(2):
                    yps = w_pool.tile([P, 512], f32, name="y_ps", tag="w_ps",
                                      bufs=4)
                    for i, fc in enumerate(fcs):
                        nc.tensor.matmul(
                            yps[:],
                            lhsT=oT[fc][:, P * tcn:P * (tcn + 1)],
                            rhs=owTp_t[:, T * fc + 512 * eh:T * fc + 512 * (eh + 1)],
                            start=(i == 0), stop=(i == len(fcs) - 1))
                    dst = ysb[:, 512 * eh:512 * (eh + 1)]
                    if not first:
                        nc.vector.tensor_add(dst, yps[:], dst)
                    elif not no_bias:
                        nc.vector.tensor_add(dst, yps[:],
                                             obb_t[:, 512 * eh:512 * (eh + 1)])
                    elif eh == 0:
                        nc.scalar.copy(dst, yps[:])
                    else:
                        nc.vector.tensor_copy(dst, yps[:])
                if last:
                    eng = (nc.sync, nc.gpsimd)[tcn % 2]
                    for eh in range(2):
                        eng.dma_start(
                            y_d[P * tcn:P * (tcn + 1),
                                512 * eh:512 * (eh + 1)],
                            ysb[:, 512 * eh:512 * (eh + 1)])
                    ysb_t.pop(tcn)

            def emit_outproj(tcn, vec_eng):
                emit_op_stage(tcn, (0, 1, 2, 3), True)

            # ---------------- schedule ----------------
            def av_items(p, h):
                items = []
                for lh in (2 * p, 2 * p + 1):
                    if slot_flags[lh]:
                        items.append(lambda p=p, h=h, lh=lh: emit_dps(p, h, lh))
                        items.append(lambda p=p, h=h, lh=lh: emit_muls(p, h, lh))
                for sc in range(8):
                    for lh in (2 * p, 2 * p + 1):
                        items.append(
                            lambda p=p, h=h, lh=lh, sc=sc: emit_av_mm(p, h, lh, sc))
                return items

            def k_items(p):
                return [lambda p=p: emit_k(p)]

            def q_items(p, h):
                return [lambda p=p, h=h: emit_q(p, h)]

            def v_items(lo, hi):
                return [lambda sc=sc: emit_v(sc) for sc in range(lo, hi)]

            def op_items(lo, hi):
                return [lambda t=t, v=(t % 2 == 0): emit_outproj(t, v)
                        for t in range(lo, hi)]

            def cb_items(p, h):
                return [lambda p=p, h=h: emit_copyback(p, h)]

            def emit_block(p, h, others):
                # both sc-units of a PSUM group emitted adjacently, so the
                # group's exp gets the full others window before the next
                # group needs the (single-buffered) scores tile back
                no = len(others)
                oi = 0
                for g in range(4):
                    emit_score(p, h, 2 * g)
                    emit_score(p, h, 2 * g + 1)
                    tgt = ((g + 1) * no) // 4
                    while oi < tgt:
                        others[oi]()
                        oi += 1
                while oi < no:
                    others[oi]()
                    oi += 1

            # prologue: clock ramp + pair-0 K/Q. All ec 0-3 accumulations
            # (needing only the _a half-tiles) run before any ec 4-7, so
            # the PE works while the _b input DMAs are still in flight.
            emit_warm()
            kps0 = [w_pool.tile([P, 512], f32, name="k_ps", tag="w_ps",
                                bufs=4) for _ in range(2)]
            qps0 = w_pool.tile([P, 512], f32, name="q_ps", tag="w_ps",
                               bufs=4)
            for eh in range(2):
                for ec in range(4 * eh, 4 * eh + 4):
                    for th in range(2):
                        nc.tensor.matmul(
                            kps0[th][:],
                            lhsT=wsl(wkTp_t, ec, 0, P),
                            rhs=xsl(ec, 512 * th, 512 * (th + 1)),
                            start=(ec == 0), stop=(ec == 7))
                    nc.tensor.matmul(
                        qps0[:], lhsT=wsl(wqTp_t, ec, 0, P),
                        rhs=xsl(ec, 0, 512),
                        start=(ec == 0), stop=(ec == 7))
            for th in range(2):
                dst = kT[0][:, 512 * th:512 * (th + 1)]
                if no_bias:
                    nc.vector.tensor_copy(dst, kps0[th][:])
                else:
                    nc.vector.tensor_scalar_add(dst, kps0[th][:],
                                                kb_t[:, 0:1])
            if no_bias:
                nc.vector.tensor_copy(qT[0][:, 0:512], qps0[:])
            else:
                nc.vector.tensor_scalar_add(qT[0][:, 0:512], qps0[:],
                                            qb_t[:, 0:1])

            emit_block(0, 0, k_items(1) + q_items(1, 0) + v_items(0, 4))
            emit_block(1, 0, k_items(2) + q_items(2, 0) + v_items(4, 8))
            emit_block(2, 0, av_items(0, 0) + cb_items(0, 0)
                       + k_items(3) + q_items(3, 0))
            emit_block(3, 0, av_items(1, 0) + cb_items(1, 0)
                       + av_items(2, 0) + cb_items(2, 0) + q_items(0, 1))
            emit_block(0, 1, av_items(3, 0) + cb_items(3, 0) + q_items(1, 1))
            emit_block(1, 1, av_items(0, 1) + cb_items(0, 1)
                       + q_items(2, 1) + op_items(0, 1))
            emit_block(2, 1, av_items(1, 1) + cb_items(1, 1)
                       + q_items(3, 1) + op_items(1, 3))
            s1_items = [lambda t=t: emit_op_stage(t, (0, 1), False)
                        for t in range(4, 8)]
            emit_block(3, 1, av_items(2, 1) + cb_items(2, 1)
                       + op_items(3, 4) + s1_items)
            # tail: head A av -> its copyback overlaps head B av; the
            # ungated fc2 accumulations fill the last copyback's
            # latency; only fc3 (gated on oT[3]) remains at the end.
            bc = x_pool.tile([P, T], f32, name="bc", tag="bc", bufs=2)
            for sc in range(8):
                emit_av_mm(3, 1, 6, sc)
            emit_cb_head(3, 1, 6, bc, nc.gpsimd)
            for sc in range(8):
                emit_av_mm(3, 1, 7, sc)
            emit_cb_head(3, 1, 7, bc, nc.gpsimd)
            for tcn in range(4, 8):
                emit_op_stage(tcn, (2, 3), True)

    nc.compile()
    return nc


def _get_program(mask_on, slot_flags, no_bias):
    key = (mask_on, slot_flags, no_bias)
    if key not in _PROGS:
        _PROGS[key] = _build_program(mask_on, slot_flags, no_bias)
    return _PROGS[key]


def _pack8(a):
    """[8*128, W] -> [128, 8*W] (chunk-major columns)."""
    n, w = a.shape[0] // P, a.shape[1]
    return np.ascontiguousarray(
        a.reshape(n, P, w).transpose(1, 0, 2).reshape(P, n * w))


def _prep_inputs(inputs):
    hs = np.asarray(inputs["hidden_states"], dtype=np.float32)
    am = np.asarray(inputs["attention_mask"], dtype=np.float32)
    rel = np.asarray(inputs["relation_inputs"])
    hm = np.asarray(inputs["heads_mask"], dtype=np.float32)
    q_w = np.asarray(inputs["q_w"], dtype=np.float32)
    q_b = np.asarray(inputs["q_b"], dtype=np.float32)
    k_w = np.asarray(inputs["k_w"], dtype=np.float32)
    k_b = np.asarray(inputs["k_b"], dtype=np.float32)
    v_w = np.asarray(inputs["v_w"], dtype=np.float32)
    v_b = np.asarray(inputs["v_b"], dtype=np.float32)
    o_w = np.asarray(inputs["o_w"], dtype=np.float32)
    o_b = np.asarray(inputs["o_b"], dtype=np.float32)

    mask_on = bool(np.any(am != 0.0))
    no_bias = not (np.any(q_b != 0.0) or np.any(k_b != 0.0)
                   or np.any(v_b != 0.0) or np.any(o_b != 0.0))
    slot_flags = tuple(
        k == 0 or bool(np.any(hm[[k, 8 + k]] != 0.0)) for k in range(8))

    relbinT = [(rel[b] > 0).T.astype(np.float32) for b in range(B)]
    if mask_on:
        expmT = [np.exp(am[b, 0]).T.astype(np.float32) for b in range(B)]

    in_maps = []
    for c in range(N_CORES):
        b, g = c // 2, c % 2
        sl = slice(FH * g, FH * (g + 1))
        im = {
            "xTp": _pack8(np.ascontiguousarray(hs[b].T)).astype(BF16),
            "wqTp": _pack8(
                np.ascontiguousarray((q_w[sl] * SCALING).T)).astype(BF16),
            "wkTp": _pack8(np.ascontiguousarray(k_w[sl].T)).astype(BF16),
            "wvTp": _pack8(np.ascontiguousarray(v_w[sl].T)).astype(BF16),
            "owTp": _pack8(np.ascontiguousarray(o_w[:, sl].T)).astype(BF16),
        }
        if not no_bias:
            im["qb"] = np.ascontiguousarray(
                (q_b[sl] * SCALING).reshape(4, P).T).astype(np.float32)
            im["kb"] = np.ascontiguousarray(
                k_b[sl].reshape(4, P).T).astype(np.float32)
            im["vbb"] = np.ascontiguousarray(
                np.broadcast_to(v_b[sl], (P, FH))).astype(np.float32)
            im["obb"] = (np.ascontiguousarray(np.broadcast_to(o_b, (P, E)))
                         .astype(np.float32) if g == 0
                         else np.zeros((P, E), np.float32))
        for k in range(8):
            if slot_flags[k]:
                hmv = float(hm[8 * g + k])
                m = (1.0 - hmv) + hmv * relbinT[b]
                im[f"relMp{k}"] = _pack8(m).astype(BF16)
        if mask_on:
            im["expmp"] = _pack8(expmT[b]).astype(BF16)
        in_maps.append(im)
    return (mask_on, slot_flags, no_bias), in_maps


def _gather(results):
    out = np.empty((B, T, E), dtype=np.float32)
    for b in range(B):
        out[b] = results[2 * b]["y"] + results[2 * b + 1]["y"]
    return out


def run_sharded(inputs, trace=False, trace_kwargs=None):
    from concourse.bass_utils import run_bass_kernel_spmd

    flags, in_maps = _prep_inputs(inputs)
    nc = _get_program(*flags)
    last_err = None
    for _attempt in range(3):
        try:
            res = run_bass_kernel_spmd(nc, in_maps, list(range(N_CORES)),
                                       trace=trace, **(trace_kwargs or {}))
            return _gather(res.results), res
        except Exception as e:  # first exec of a fresh NEFF can flake
            last_err = e
    raise last_err


def kernel(**inputs):
    out, _ = run_sharded(inputs)
    return out
